# revision 1
# baseline (speedup 1.0000x reference)
"""Trainium2 Bass kernel for nn_ConvBlock (conv1d x3 + per-subject BN + GELU).

Sharding: data-parallel over batch across 8 NeuronCores (32 items/core).
Per-subject BN stats are reduced across cores with an in-kernel AllReduce
of (sum, sumsq) per (subject, channel); counts are host-known constants.

Self-contained: shapes hardcoded, no sibling imports.
"""

import os
import sys
import types

import numpy as np

# ---------------------------------------------------------------- constants
B, CIN, COUT, T = 256, 271, 320, 512
S = 4  # subjects
NCORES = 8
BSH = B // NCORES  # 32 items per core
EPS = 1e-5
NRES = 16  # items per core whose activations stay SBUF-resident across stages

# channel tiling (partition dim is 128)
KT0 = [(0, 128), (128, 256), (256, CIN)]  # conv0 contraction tiles (271)
KT = [(0, 128), (128, 256), (256, COUT)]  # conv1/2 contraction tiles (320)
CT = [(0, 128), (128, 256), (256, COUT)]  # output-channel tiles (320)

_F32 = None  # filled lazily (mybir.dt.float32)


def _install_ntff_hook():
    """Optionally enable NTFF profiling under axon (for tracing only)."""
    try:
        if "antenv.axon_hooks" not in sys.modules:
            import antenv  # noqa: F401

            mod = types.ModuleType("antenv.axon_hooks")
            _hook = [None]
            mod.set_axon_ntff_profile_hook = lambda h: _hook.__setitem__(0, h)
            mod.get_axon_ntff_profile_hook = lambda: _hook[0]
            sys.modules["antenv.axon_hooks"] = mod
            antenv.axon_hooks = mod
        from antenv.axon_hooks import (
            get_axon_ntff_profile_hook,
            set_axon_ntff_profile_hook,
        )

        if get_axon_ntff_profile_hook() is None:
            from trn_agent_boot.trn_boot import _ntff_profile_via_ctypes

            set_axon_ntff_profile_hook(
                _ntff_profile_via_ctypes("/opt/axon/libaxon_pjrt.so")
            )
    except Exception:
        pass


def _split_multi_waits(nc, mybir):
    """This env's walrus accepts one sync-wait per instruction: hoist extras
    onto separate same-engine nops placed just before the instruction."""
    for f in nc.m.functions:
        for bb in f.blocks:
            insts = list(bb.instructions)
            out = []
            changed = False
            for inst in insts:
                si = inst.sync_info
                if si is not None and si.on_wait and len(si.on_wait) > 1:
                    waits = list(si.on_wait)
                    for w in waits[:-1]:
                        d = mybir.InstNoOp(
                            name=nc.get_next_instruction_name(), ins=[], outs=[]
                        )
                        d.engine = inst.engine
                        d.sync_info = mybir.SyncInfo(on_wait=[w], on_update=[])
                        nc.register_instruction(d)
                        out.append(d)
                    inst.sync_info = mybir.SyncInfo(
                        on_wait=[waits[-1]], on_update=list(si.on_update or [])
                    )
                    changed = True
                out.append(inst)
            if changed:
                bb.instructions[:] = out


def _build_program():
    import concourse.bass as bass
    import concourse.mybir as mybir
    from concourse import tile

    F32 = mybir.dt.float32
    F32R = mybir.dt.float32r
    ADD = mybir.AluOpType.add
    MULT = mybir.AluOpType.mult
    SUB = mybir.AluOpType.subtract
    GELU = mybir.ActivationFunctionType.Gelu
    SQRT = mybir.ActivationFunctionType.Sqrt

    nc = bass.Bass("TRN2", target_bir_lowering=False, debug=False, num_devices=NCORES)

    # ---------------- I/O ----------------
    Xd = nc.dram_tensor("xsh", [BSH, CIN, T], F32, kind="ExternalInput").ap()
    wts = {}
    for s_i, cin in ((0, CIN), (1, COUT), (2, COUT)):
        for tap in range(3):
            wts[(s_i, tap)] = nc.dram_tensor(
                f"w{s_i}t{tap}", [cin, COUT], F32, kind="ExternalInput"
            ).ap()
    wtaild = {
        0: nc.dram_tensor("w0tail", [96, COUT], F32, kind="ExternalInput").ap(),
        1: nc.dram_tensor("w1tail", [128, COUT], F32, kind="ExternalInput").ap(),
        2: nc.dram_tensor("w2tail", [128, COUT], F32, kind="ExternalInput").ap(),
    }
    masksd = nc.dram_tensor("masks", [S, 128, BSH], F32, kind="ExternalInput").ap()
    invcd = nc.dram_tensor("invc", [128, S], F32, kind="ExternalInput").ap()
    gcmd = [
        nc.dram_tensor(f"gcm{s_i}", [COUT, S], F32, kind="ExternalInput").ap()
        for s_i in range(3)
    ]
    becmd = [
        nc.dram_tensor(f"becm{s_i}", [COUT, S], F32, kind="ExternalInput").ap()
        for s_i in range(3)
    ]
    OUTd = nc.dram_tensor("out", [BSH, COUT, T], F32, kind="ExternalOutput").ap()

    # DRAM scratch arenas (per-stage activations) + collective bounces
    Yd = [
        nc.dram_tensor(f"y{s_i}", [BSH, COUT, T], F32).ap() for s_i in range(3)
    ]
    ccin = [nc.dram_tensor(f"ccin{s_i}", [128, 24], F32).ap() for s_i in range(3)]
    ccout = [nc.dram_tensor(f"ccout{s_i}", [128, 24], F32).ap() for s_i in range(3)]

    with tile.TileContext(nc) as tc:
        with (
            tc.tile_pool(name="consts", bufs=1) as cpool,
            tc.tile_pool(name="wstag", bufs=2) as wstag,
            tc.tile_pool(name="zr", bufs=8) as zpool,
            tc.tile_pool(name="yin", bufs=6) as yinpool,
            tc.tile_pool(name="ynew", bufs=6) as ynpool,
            tc.tile_pool(name="sq", bufs=2) as sqpool,
            tc.tile_pool(name="isums", bufs=1) as ispool,
            tc.tile_pool(name="small", bufs=1) as smpool,
            tc.tile_pool(name="scsh", bufs=1) as scpool,
            tc.tile_pool(name="sctmp", bufs=4) as sctpool,
            tc.tile_pool(name="yres", bufs=54) as yrpool,
            tc.tile_pool(name="ztail", bufs=4) as ztpool,
            tc.tile_pool(name="psum", bufs=6, space="PSUM") as pspool,
        ):
            # ---------------- load constants ----------------
            mask_t = []
            for s in range(S):
                mt = cpool.tile([128, BSH], F32, name=f"mask{s}")
                nc.sync.dma_start(mt[:], masksd[s])
                mask_t.append(mt)
            invc_t = cpool.tile([128, S], F32, name="invct")
            nc.sync.dma_start(invc_t[:], invcd[:])
            gcm_t = []  # [stage][ct] -> [128,4]
            becm_t = []
            for s_i in range(3):
                gl, bl = [], []
                for ci, (c0, c1) in enumerate(CT):
                    m = c1 - c0
                    g = cpool.tile([128, S], F32, name=f"g{s_i}_{ci}")
                    bt = cpool.tile([128, S], F32, name=f"b{s_i}_{ci}")
                    nc.sync.dma_start(g[:m, :], gcmd[s_i][c0:c1, :])
                    nc.sync.dma_start(bt[:m, :], becmd[s_i][c0:c1, :])
                    gl.append(g)
                    bl.append(bt)
                gcm_t.append(gl)
                becm_t.append(bl)

            # weights -> f32r tiles (full 128-row k-tiles per tap + stacked tails)
            wr = {}  # (stage, kt, tap) -> [128, COUT] f32r tile
            wtail_r = {}
            for s_i in range(3):
                for ki in (0, 1):
                    k0, k1 = ki * 128, (ki + 1) * 128
                    for tap in range(3):
                        stg = wstag.tile([128, COUT], F32, name="wstg")
                        nc.sync.dma_start(stg[:, :], wts[(s_i, tap)][k0:k1, :])
                        wt = cpool.tile(
                            [128, COUT], F32R, name=f"wr{s_i}_{ki}_{tap}"
                        )
                        nc.vector.tensor_copy(wt[:, :], stg[:, :])
                        wr[(s_i, ki, tap)] = wt
                tsz = 96 if s_i == 0 else 128
                stg = wstag.tile([128, COUT], F32, name="wstg")
                nc.sync.dma_start(stg[:tsz, :], wtaild[s_i][:, :])
                wt = cpool.tile([128, COUT], F32R, name=f"wtail{s_i}")
                nc.vector.tensor_copy(wt[:tsz, :], stg[:tsz, :])
                wtail_r[s_i] = wt
                if s_i > 0:
                    # last tap of the 64-ch tail stays a separate K=64 matmul
                    stg2 = wstag.tile([128, COUT], F32, name="wstg")
                    nc.sync.dma_start(stg2[:64, :], wts[(s_i, 2)][256:320, :])
                    wt2 = cpool.tile([128, COUT], F32R, name=f"wtail2_{s_i}")
                    nc.vector.tensor_copy(wt2[:64, :], stg2[:64, :])
                    wr[(s_i, 2, 2)] = wt2
                else:
                    pass

            # per-item bn-apply scale/shift tiles, per stage
            SC = [None, None, None]  # stage -> [ct] -> [128, BSH]
            SH = [None, None, None]

            def conv_stage(s_i, ylive=None):
                """One full stage: (apply prev bn+gelu ->) conv -> sums; then
                stats allreduce -> per-item scale/shift for next apply."""
                ktiles = KT0 if s_i == 0 else KT
                ynext = [[None] * 3 for _ in range(NRES)]
                i1 = [ispool.tile([128, BSH], F32, name=f"i1_{s_i}_{c}") for c in range(3)]
                i2 = [ispool.tile([128, BSH], F32, name=f"i2_{s_i}_{c}") for c in range(3)]

                for b in range(BSH):
                    # ---- produce conv input z (f32r) ----
                    zr = []
                    tailsz = ktiles[2][1] - ktiles[2][0]
                    for ki, (k0, k1) in enumerate(ktiles):
                        ksz = k1 - k0
                        zt = zpool.tile([128, T + 4], F32R, name="zr")
                        if s_i == 0 and b < 4:
                            nc.vector.memset(zt[:, 0:2].bitcast(F32), 0.0)
                            nc.vector.memset(zt[:, T + 2 : T + 4].bitcast(F32), 0.0)
                        if s_i == 0:
                            xst = yinpool.tile([128, T], F32, name="yin")
                            nc.sync.dma_start(xst[:ksz, :], Xd[b, k0:k1, :])
                            nc.vector.tensor_copy(zt[:ksz, 2 : T + 2], xst[:ksz, :])
                        else:
                            if b < NRES:
                                yin = ylive[b][ki]
                            else:
                                yin = yinpool.tile([128, T], F32, name="yin")
                                nc.sync.dma_start(
                                    yin[:ksz, :], Yd[s_i - 1][b, k0:k1, :]
                                )
                            nc.scalar.activation(
                                zt[:ksz, 2 : T + 2],
                                yin[:ksz, :],
                                GELU,
                                bias=SH[s_i - 1][ki][:ksz, b : b + 1],
                                scale=SC[s_i - 1][ki][:ksz, b : b + 1],
                            )
                        zr.append(zt)

                    # ---- stacked tail rhs (taps share it across couttiles) ----
                    zt3 = ztpool.tile([128, T], F32R, name="ztail")
                    if s_i == 0:
                        # taps at 32-aligned partition bases; gap rows zeroed
                        if b < 4:
                            nc.vector.memset(zt3[0:96, :].bitcast(F32), 0.0)
                        for ti in range(3):
                            nc.vector.tensor_copy(
                                zt3[32 * ti : 32 * ti + 15, :],
                                zr[2][:15, 1 + ti : 1 + ti + T].bitcast(F32),
                            )
                        tail_k = 96
                    else:
                        for ti in range(2):  # taps 0,1 x 64 ch -> K=128
                            nc.vector.tensor_copy(
                                zt3[64 * ti : 64 * ti + 64, :],
                                zr[2][:64, 1 + ti : 1 + ti + T].bitcast(F32),
                            )
                        tail_k = 128

                    # ---- conv: per couttile: 6 full-ktile taps + packed tail ----
                    for ci, (c0, c1) in enumerate(CT):
                        m = c1 - c0
                        ps = pspool.tile([128, T], F32, name="ps")
                        first = True
                        n_mm = 7 if s_i == 0 else 8
                        done = 0
                        for ki in (0, 1):
                            for tap in (1, 0, 2):
                                w = wr[(s_i, ki, tap)][:, c0:c1]
                                off = 2 + (tap - 1)  # 1 / 2 / 3
                                r_ap = zr[ki][:, off : off + T]
                                done += 1
                                nc.tensor.matmul(
                                    ps[:m, 0:T],
                                    w,
                                    r_ap,
                                    start=first,
                                    stop=(done == n_mm),
                                    skip_group_check=not first,
                                )
                                first = False
                        done += 1
                        nc.tensor.matmul(
                            ps[:m, 0:T],
                            wtail_r[s_i][:tail_k, c0:c1],
                            zt3[:tail_k, 0:T],
                            start=False,
                            stop=(done == n_mm),
                            skip_group_check=True,
                        )
                        if s_i > 0:
                            done += 1
                            nc.tensor.matmul(
                                ps[:m, 0:T],
                                wr[(s_i, 2, 2)][:64, c0:c1],
                                zr[2][:64, 3 : 3 + T],
                                start=False,
                                stop=(done == n_mm),
                                skip_group_check=True,
                            )

                        # ---- y = psum (+ residual z); per-item channel sums ----
                        if b < NRES:
                            yt = yrpool.tile([128, T], F32, name="yres")
                        else:
                            yt = ynpool.tile([128, T], F32, name="ynew")
                        if s_i == 0:
                            nc.vector.tensor_scalar(
                                out=yt[:m, :],
                                in0=ps[:m, :],
                                scalar1=1.0,
                                scalar2=0.0,
                                op0=MULT,
                                op1=ADD,
                                accum_out=i1[ci][:m, b : b + 1],
                            )
                        else:
                            nc.vector.scalar_tensor_tensor(
                                out=yt[:m, :],
                                in0=ps[:m, :],
                                scalar=0.0,
                                in1=zr[ci][:m, 2 : T + 2].bitcast(F32),
                                op0=ADD,
                                op1=ADD,
                                accum_out=i1[ci][:m, b : b + 1],
                            )
                        sq = sqpool.tile([128, T], F32, name="sq")
                        nc.vector.scalar_tensor_tensor(
                            out=sq[:m, :],
                            in0=yt[:m, :],
                            scalar=1.0,
                            in1=yt[:m, :],
                            op0=MULT,
                            op1=MULT,
                            accum_out=i2[ci][:m, b : b + 1],
                        )
                        if b < NRES:
                            ynext[b][ci] = yt
                        else:
                            nc.sync.dma_start(Yd[s_i][b, c0:c1, :], yt[:m, :])

                # ---------------- stats: mask-reduce + AllReduce ----------------
                cc = smpool.tile([128, 24], F32, name=f"cc{s_i}")
                scr = sctpool.tile([128, BSH], F32, name="scr")
                for ci in range(3):
                    for s in range(S):
                        nc.vector.scalar_tensor_tensor(
                            out=scr[:, :],
                            in0=i1[ci][:, :],
                            scalar=1.0,
                            in1=mask_t[s][:, :],
                            op0=MULT,
                            op1=MULT,
                            accum_out=cc[:, ci * 4 + s : ci * 4 + s + 1],
                        )
                        nc.vector.scalar_tensor_tensor(
                            out=scr[:, :],
                            in0=i2[ci][:, :],
                            scalar=1.0,
                            in1=mask_t[s][:, :],
                            op0=MULT,
                            op1=MULT,
                            accum_out=cc[:, 12 + ci * 4 + s : 12 + ci * 4 + s + 1],
                        )
                nc.sync.dma_start(ccin[s_i][:, :], cc[:, :])
                nc.gpsimd.collective_compute(
                    "AllReduce",
                    ADD,
                    replica_groups=[list(range(NCORES))],
                    ins=[ccin[s_i][:, :]],
                    outs=[ccout[s_i][:, :]],
                )
                gsb = smpool.tile([128, 24], F32, name=f"gsb{s_i}")
                nc.sync.dma_start(gsb[:, :], ccout[s_i][:, :])

                # ---------------- scale/shift per (ct, subject) ----------------
                SCs, SHs = [], []
                for ci, (c0, c1) in enumerate(CT):
                    m = c1 - c0
                    g1 = gsb[:, ci * 4 : ci * 4 + 4]
                    g2 = gsb[:, 12 + ci * 4 : 12 + ci * 4 + 4]
                    mean = smpool.tile([128, S], F32, name="mean")
                    nc.vector.tensor_tensor(
                        out=mean[:, :], in0=g1, in1=invc_t[:, :], op=MULT
                    )
                    var = smpool.tile([128, S], F32, name="var")
                    # ex2 = g2*invc ; var = ex2 - mean*mean
                    nc.vector.tensor_tensor(
                        out=var[:, :], in0=g2, in1=invc_t[:, :], op=MULT
                    )
                    msq = smpool.tile([128, S], F32, name="msq")
                    nc.vector.scalar_tensor_tensor(
                        out=msq[:, :],
                        in0=mean[:, :],
                        scalar=1.0,
                        in1=mean[:, :],
                        op0=MULT,
                        op1=MULT,
                    )
                    nc.vector.tensor_tensor(
                        out=var[:, :], in0=var[:, :], in1=msq[:, :], op=SUB
                    )
                    nc.vector.tensor_scalar_add(var[:, :], var[:, :], EPS)
                    std = smpool.tile([128, S], F32, name="std")
                    nc.scalar.activation(std[:, :], var[:, :], SQRT)
                    rinv = smpool.tile([128, S], F32, name="rinv")
                    nc.vector.reciprocal(rinv[:, :], std[:, :])
                    scale = smpool.tile([128, S], F32, name="scale")
                    nc.vector.tensor_tensor(
                        out=scale[:, :], in0=rinv[:, :], in1=gcm_t[s_i][ci][:, :], op=MULT
                    )
                    shift = smpool.tile([128, S], F32, name="shift")
                    nc.vector.scalar_tensor_tensor(
                        out=shift[:, :],
                        in0=mean[:, :],
                        scalar=1.0,
                        in1=scale[:, :],
                        op0=MULT,
                        op1=MULT,
                    )
                    nc.vector.tensor_tensor(
                        out=shift[:, :],
                        in0=becm_t[s_i][ci][:, :],
                        in1=shift[:, :],
                        op=SUB,
                    )

                    # expand subject -> per-item columns via masks
                    sct = scpool.tile([128, BSH], F32, name=f"SC{s_i}_{ci}")
                    sht = scpool.tile([128, BSH], F32, name=f"SH{s_i}_{ci}")
                    for dst, src in ((sct, scale), (sht, shift)):
                        prev = None
                        for s in range(S):
                            o = dst if s == S - 1 else sctpool.tile(
                                [128, BSH], F32, name="acc"
                            )
                            if prev is None:
                                nc.vector.tensor_scalar_mul(
                                    o[:, :], mask_t[s][:, :], src[:, s : s + 1]
                                )
                            else:
                                nc.vector.scalar_tensor_tensor(
                                    out=o[:, :],
                                    in0=mask_t[s][:, :],
                                    scalar=src[:, s : s + 1],
                                    in1=prev[:, :],
                                    op0=MULT,
                                    op1=ADD,
                                )
                            prev = o
                    SCs.append(sct)
                    SHs.append(sht)
                SC[s_i] = SCs
                SH[s_i] = SHs
                return ynext

            ylive = conv_stage(0)
            ylive = conv_stage(1, ylive)
            ylive = conv_stage(2, ylive)

            # ---------------- final apply: out = gelu(bn2(y2)) ----------------
            for b in range(BSH):
                for ci, (c0, c1) in enumerate(CT):
                    m = c1 - c0
                    if b < NRES:
                        yin = ylive[b][ci]
                    else:
                        yin = yinpool.tile([128, T], F32, name="yin")
                        nc.sync.dma_start(yin[:m, :], Yd[2][b, c0:c1, :])
                    zo = ynpool.tile([128, T], F32, name="ynew")
                    nc.scalar.activation(
                        zo[:m, :],
                        yin[:m, :],
                        GELU,
                        bias=SH[2][ci][:m, b : b + 1],
                        scale=SC[2][ci][:m, b : b + 1],
                    )
                    nc.sync.dma_start(OUTd[b, c0:c1, :], zo[:m, :])

    _split_multi_waits(nc, mybir)
    return nc


_CACHED = {}


def kernel(**inputs):
    X = np.ascontiguousarray(np.asarray(inputs["X"], dtype=np.float32))
    subj = np.asarray(inputs["subject_idxs"], dtype=np.int32)
    w = [np.asarray(inputs[f"w{i}"], dtype=np.float32) for i in range(3)]
    g = [np.asarray(inputs[k], dtype=np.float32) for k in ("g0", "g1", "g2")]
    be = [np.asarray(inputs[k], dtype=np.float32) for k in ("be0", "be1", "be2")]
    # biases cancel inside per-subject BN (uniform per-channel shift is
    # absorbed by the per-subject mean), so b0/b1/b2 are not needed.

    from concourse.bass_utils import run_bass_kernel_spmd

    trace = bool(int(os.environ.get("BASS_KERNEL_TRACE", "0")))
    if trace:
        _install_ntff_hook()

    if "nc" not in _CACHED:
        _CACHED["nc"] = _build_program()
    nc = _CACHED["nc"]

    # ---------------- host-side prep ----------------
    cnt = np.maximum(
        np.bincount(subj, minlength=S).astype(np.float32) * float(T), 1.0
    )
    invc = np.broadcast_to((1.0 / cnt)[None, :], (128, S)).copy()

    shared = {"invc": invc}
    for s_i in range(3):
        for tap in range(3):
            shared[f"w{s_i}t{tap}"] = np.ascontiguousarray(w[s_i][:, :, tap].T)
        wT = [np.ascontiguousarray(w[s_i][:, :, tap].T) for tap in range(3)]
        if s_i == 0:
            wt0 = np.zeros((96, COUT), dtype=np.float32)
            for ti in range(3):
                wt0[32 * ti : 32 * ti + 15] = wT[ti][256:271]
            shared["w0tail"] = wt0
        else:
            shared[f"w{s_i}tail"] = np.ascontiguousarray(
                np.concatenate([wT[0][256:320], wT[1][256:320]], axis=0)
            )
        shared[f"gcm{s_i}"] = np.ascontiguousarray(g[s_i].T)  # [COUT, S]
        shared[f"becm{s_i}"] = np.ascontiguousarray(be[s_i].T)

    in_maps = []
    for c in range(NCORES):
        sl = slice(c * BSH, (c + 1) * BSH)
        subj_c = subj[sl]
        masks = np.zeros((S, 128, BSH), dtype=np.float32)
        for bi in range(BSH):
            masks[subj_c[bi], :, bi] = 1.0
        m = dict(shared)
        m["xsh"] = X[sl]
        m["masks"] = masks
        in_maps.append(m)

    res = run_bass_kernel_spmd(
        nc, in_maps, core_ids=list(range(NCORES)), trace=trace
    )
    if trace:
        _CACHED["exec_time_ns"] = res.exec_time_ns
        _CACHED["results_obj"] = res

    out = np.empty((B, COUT, T), dtype=np.float32)
    for c in range(NCORES):
        out[c * BSH : (c + 1) * BSH] = res.results[c]["out"]
    return out



# revision 9
# speedup vs baseline: 1.2354x; 1.2354x over previous
"""Trainium2 Bass kernel for nn_ConvBlock (conv1d x3 + per-subject BN + GELU).

Sharding: data-parallel over batch across 8 NeuronCores (32 items/core).
Per-subject BN stats are reduced across cores with an in-kernel AllReduce
of (sum, sumsq) per (subject, channel); counts are host-known constants.

All activations and weights are fp16 (PSUM/stats fp32): fp16 matmuls
stream at 1 col/cycle with LDWEIGHTS hidden, activations stay fully
SBUF-resident (no HBM spills), and X/OUT HBM traffic is halved.
Measured end-to-end accuracy of the fp16 pipeline: ~1e-3 max rel err.

Self-contained: shapes hardcoded, no sibling imports.
"""

import os
import sys
import types

import numpy as np

# ---------------------------------------------------------------- constants
B, CIN, COUT, T = 256, 271, 320, 512
S = 4  # subjects
NCORES = 8
BSH = B // NCORES  # 32 items per core
EPS = 1e-5
CT = [(0, 128), (128, 256), (256, COUT)]  # output-channel tiles


def _install_ntff_hook():
    """Optionally enable NTFF profiling under axon (for tracing only)."""
    try:
        if "antenv.axon_hooks" not in sys.modules:
            import antenv  # noqa: F401

            mod = types.ModuleType("antenv.axon_hooks")
            _hook = [None]
            mod.set_axon_ntff_profile_hook = lambda h: _hook.__setitem__(0, h)
            mod.get_axon_ntff_profile_hook = lambda: _hook[0]
            sys.modules["antenv.axon_hooks"] = mod
            antenv.axon_hooks = mod
        from antenv.axon_hooks import (
            get_axon_ntff_profile_hook,
            set_axon_ntff_profile_hook,
        )

        if get_axon_ntff_profile_hook() is None:
            from trn_agent_boot.trn_boot import _ntff_profile_via_ctypes

            set_axon_ntff_profile_hook(
                _ntff_profile_via_ctypes("/opt/axon/libaxon_pjrt.so")
            )
    except Exception:
        pass


def _split_multi_waits(nc, mybir):
    """This env's walrus accepts one sync-wait per instruction: hoist extras
    onto separate same-engine nops placed just before the instruction."""
    for f in nc.m.functions:
        for bb in f.blocks:
            insts = list(bb.instructions)
            out = []
            changed = False
            for inst in insts:
                si = inst.sync_info
                if si is not None and si.on_wait and len(si.on_wait) > 1:
                    waits = list(si.on_wait)
                    for w in waits[:-1]:
                        d = mybir.InstNoOp(
                            name=nc.get_next_instruction_name(), ins=[], outs=[]
                        )
                        d.engine = inst.engine
                        d.sync_info = mybir.SyncInfo(on_wait=[w], on_update=[])
                        nc.register_instruction(d)
                        out.append(d)
                    inst.sync_info = mybir.SyncInfo(
                        on_wait=[waits[-1]], on_update=list(si.on_update or [])
                    )
                    changed = True
                out.append(inst)
            if changed:
                bb.instructions[:] = out


def _build_program():
    import concourse.bass as bass
    import concourse.mybir as mybir
    from concourse import tile

    F16 = mybir.dt.float16
    F32 = mybir.dt.float32
    ADD = mybir.AluOpType.add
    MULT = mybir.AluOpType.mult
    SUB = mybir.AluOpType.subtract
    GELU = mybir.ActivationFunctionType.Gelu
    SQRT = mybir.ActivationFunctionType.Sqrt

    nc = bass.Bass("TRN2", target_bir_lowering=False, debug=False, num_devices=NCORES)

    # ---------------- I/O ----------------
    Xd = nc.dram_tensor("xsh", [BSH, CIN, T], F16, kind="ExternalInput").ap()
    Wd = nc.dram_tensor("wpk", [23, 128, COUT], F16, kind="ExternalInput").ap()
    masksd = nc.dram_tensor("masks", [S, 128, BSH], F32, kind="ExternalInput").ap()
    invcd = nc.dram_tensor("invc", [128, S], F32, kind="ExternalInput").ap()
    gcmd = nc.dram_tensor("gcm", [3, 3, 128, S], F32, kind="ExternalInput").ap()
    becmd = nc.dram_tensor("becm", [3, 3, 128, S], F32, kind="ExternalInput").ap()
    OUTd = nc.dram_tensor("out", [BSH, COUT, T], F16, kind="ExternalOutput").ap()
    ccin = [nc.dram_tensor(f"ccin{s}", [128, 24], F32).ap() for s in range(3)]
    ccout = [nc.dram_tensor(f"ccout{s}", [128, 24], F32).ap() for s in range(3)]

    with tile.TileContext(nc) as tc:
        with (
            tc.tile_pool(name="main", bufs=1) as mp,
            tc.tile_pool(name="psum", bufs=1, space="PSUM") as pp,
        ):
            # ---------------- constants ----------------
            wt = []
            for i in range(23):
                w = mp.tile([128, COUT], F16, name=f"wt{i}")
                nc.sync.dma_start(w[:, :], Wd[i])
                wt.append(w)
            mask_t = []
            for s in range(S):
                m = mp.tile([128, BSH], F32, name=f"mask{s}")
                nc.sync.dma_start(m[:, :], masksd[s])
                mask_t.append(m)
            invc_t = mp.tile([128, S], F32, name="invct")
            nc.sync.dma_start(invc_t[:, :], invcd[:, :])
            gcm_t, becm_t = [], []
            for s in range(3):
                gl, bl = [], []
                for ci in range(3):
                    g = mp.tile([128, S], F32, name=f"g{s}_{ci}")
                    bt = mp.tile([128, S], F32, name=f"b{s}_{ci}")
                    nc.sync.dma_start(g[:, :], gcmd[s, ci])
                    nc.sync.dma_start(bt[:, :], becmd[s, ci])
                    gl.append(g)
                    bl.append(bt)
                gcm_t.append(gl)
                becm_t.append(bl)

            # ---------------- working buffers (explicit ref cycling) -----
            TP = T + 4  # padded z width: col j holds z[j-1], cols 0/513 zero
            zAb = [mp.tile([128, TP], F16, name=f"zA{i}") for i in range(4)]
            zBb = [mp.tile([128, TP], F16, name=f"zB{i}") for i in range(4)]
            zCb = [mp.tile([128, TP], F16, name=f"zC{i}") for i in range(2)]
            ztl = [mp.tile([128, T], F16, name=f"ztl{i}") for i in range(4)]
            zt0 = [mp.tile([96, TP], F16, name=f"zt0{i}") for i in range(4)]
            sqb = [mp.tile([128, T], F16, name=f"sq{i}") for i in range(6)]
            oA = [mp.tile([128, T], F16, name=f"oA{i}") for i in range(4)]
            oB = [mp.tile([128, T], F16, name=f"oB{i}") for i in range(4)]
            oC = [mp.tile([128, T], F16, name=f"oC{i}") for i in range(4)]
            scr = [mp.tile([128, BSH], F32, name=f"scr{i}") for i in range(4)]
            ps = [pp.tile([128, T], F32, name=f"ps{i}") for i in range(6)]

            yA = [mp.tile([128, T], F16, name=f"yA{b}") for b in range(BSH)]
            yB = [mp.tile([128, T], F16, name=f"yB{b}") for b in range(BSH)]
            yC = [mp.tile([128, T], F16, name=f"yC{p}") for p in range(BSH // 2)]

            i1 = [[mp.tile([128, BSH], F32, name=f"i1_{s}_{c}") for c in range(3)]
                  for s in range(3)]
            i2 = [[mp.tile([128, BSH], F32, name=f"i2_{s}_{c}") for c in range(3)]
                  for s in range(3)]
            SC = [[mp.tile([128, BSH], F32, name=f"SC{s}_{c}") for c in range(3)]
                  for s in range(3)]
            SH = [[mp.tile([128, BSH], F32, name=f"SH{s}_{c}") for c in range(3)]
                  for s in range(3)]

            # zero halos once (producers never write cols 0 / T+1)
            for z in zAb + zBb + zCb:
                nc.vector.memset(z[:, 0:1], 0.0)
                nc.vector.memset(z[:, T + 1:TP], 0.0)
            # stage0 tail pack: taps at 32-aligned partition bases. Zero the
            # whole tile: gap rows have zero weights, but 0*garbage-NaN would
            # still poison PSUM, and edge columns must read as zero padding.
            for z in zt0:
                nc.vector.memset(z[0:96, :], 0.0)

            def conv_item(s, b):
                """Matmuls + y/stat passes for one item in stage s."""
                h = 64 * (b % 2)
                zA, zB = zAb[b % 4], zBb[b % 4]
                zC = zCb[(b // 2) % 2]
                n_mm = 7 if s == 0 else 8
                for ci, (c0, c1) in enumerate(CT):
                    mm = c1 - c0
                    p = ps[(3 * b + ci) % 6]
                    pout = p[h:h + 64, 0:T] if ci == 2 else p[0:mm, 0:T]
                    k = 0
                    for kt in (0, 1):
                        zt_ = zA if kt == 0 else zB
                        for tap in range(3):
                            nc.tensor.matmul(
                                pout,
                                wt[s * 6 + kt * 3 + tap][:, c0:c1],
                                zt_[0:128, tap:tap + T],
                                start=(k == 0),
                                stop=(k == n_mm - 1),
                                skip_group_check=(k > 0),
                            )
                            k += 1
                    if s == 0:
                        nc.tensor.matmul(
                            pout, wt[18][0:96, c0:c1], zt0[b % 4][0:96, 0:T],
                            start=False, stop=True, skip_group_check=True)
                    else:
                        base = 19 + 2 * (s - 1)
                        nc.tensor.matmul(
                            pout, wt[base][0:128, c0:c1], ztl[b % 4][0:128, 0:T],
                            start=False, stop=False, skip_group_check=True)
                        nc.tensor.matmul(
                            pout, wt[base + 1][h:h + 64, c0:c1],
                            zC[h:h + 64, 2:2 + T],
                            start=False, stop=True, skip_group_check=True)

                    # y = psum (+ residual z); accumulate per-item sums
                    if ci == 2:
                        yt_ap = yC[b // 2][h:h + 64, 0:T]
                        p_ap = p[h:h + 64, 0:T]
                        a1 = i1[s][2][h:h + 64, b:b + 1]
                        a2 = i2[s][2][h:h + 64, b:b + 1]
                        zres = zC[h:h + 64, 1:1 + T]
                        sq_ap = sqb[(3 * b + ci) % 6][h:h + 64, 0:T]
                    else:
                        yt = yA[b] if ci == 0 else yB[b]
                        yt_ap = yt[0:128, 0:T]
                        p_ap = p[0:128, 0:T]
                        a1 = i1[s][ci][:, b:b + 1]
                        a2 = i2[s][ci][:, b:b + 1]
                        zres = (zA if ci == 0 else zB)[0:128, 1:1 + T]
                        sq_ap = sqb[(3 * b + ci) % 6][0:128, 0:T]
                    if s == 0:
                        nc.vector.tensor_scalar(
                            out=yt_ap, in0=p_ap, scalar1=1.0, scalar2=0.0,
                            op0=MULT, op1=ADD, accum_out=a1)
                    else:
                        nc.vector.scalar_tensor_tensor(
                            out=yt_ap, in0=p_ap, scalar=1.0, in1=zres,
                            op0=MULT, op1=ADD, accum_out=a1)
                    nc.vector.scalar_tensor_tensor(
                        out=sq_ap, in0=yt_ap, scalar=1.0, in1=yt_ap,
                        op0=MULT, op1=MULT, accum_out=a2)

            def stats_stage(s):
                """Mask-reduce per-item sums, AllReduce, per-(ct,subj)
                scale/shift, expand to per-item columns."""
                cc = mp.tile([128, 24], F32, name=f"cc{s}")
                for ci in range(3):
                    for sj in range(S):
                        nc.vector.scalar_tensor_tensor(
                            out=scr[sj % 4][:, :], in0=i1[s][ci][:, :],
                            scalar=1.0, in1=mask_t[sj][:, :],
                            op0=MULT, op1=MULT,
                            accum_out=cc[:, ci * 4 + sj:ci * 4 + sj + 1])
                        nc.vector.scalar_tensor_tensor(
                            out=scr[sj % 4][:, :], in0=i2[s][ci][:, :],
                            scalar=1.0, in1=mask_t[sj][:, :],
                            op0=MULT, op1=MULT,
                            accum_out=cc[:, 12 + ci * 4 + sj:12 + ci * 4 + sj + 1])
                nc.sync.dma_start(ccin[s][:, :], cc[:, :])
                nc.gpsimd.collective_compute(
                    "AllReduce", ADD,
                    replica_groups=[list(range(NCORES))],
                    ins=[ccin[s][:, :]], outs=[ccout[s][:, :]])
                gsb = mp.tile([128, 24], F32, name=f"gsb{s}")
                nc.sync.dma_start(gsb[:, :], ccout[s][:, :])

                # ct2 columns: even items acc on rows 0-63, odd on 64-127;
                # fold halves then duplicate so both halves hold the totals
                ccf = mp.tile([128, 8], F32, name=f"ccf{s}")
                nc.sync.dma_start(ccf[0:64, 0:4], gsb[64:128, 8:12])
                nc.sync.dma_start(ccf[0:64, 4:8], gsb[64:128, 20:24])
                nc.vector.tensor_tensor(
                    out=gsb[0:64, 8:12], in0=gsb[0:64, 8:12],
                    in1=ccf[0:64, 0:4], op=ADD)
                nc.vector.tensor_tensor(
                    out=gsb[0:64, 20:24], in0=gsb[0:64, 20:24],
                    in1=ccf[0:64, 4:8], op=ADD)
                nc.vector.tensor_copy(gsb[64:128, 8:12], gsb[0:64, 8:12])
                nc.vector.tensor_copy(gsb[64:128, 20:24], gsb[0:64, 20:24])

                for ci in range(3):
                    g1 = gsb[:, ci * 4:ci * 4 + 4]
                    g2 = gsb[:, 12 + ci * 4:12 + ci * 4 + 4]
                    mean = mp.tile([128, S], F32, name=f"mean{s}_{ci}")
                    nc.vector.tensor_tensor(
                        out=mean[:, :], in0=g1, in1=invc_t[:, :], op=MULT)
                    var = mp.tile([128, S], F32, name=f"var{s}_{ci}")
                    nc.vector.tensor_tensor(
                        out=var[:, :], in0=g2, in1=invc_t[:, :], op=MULT)
                    msq = mp.tile([128, S], F32, name=f"msq{s}_{ci}")
                    nc.vector.tensor_tensor(
                        out=msq[:, :], in0=mean[:, :], in1=mean[:, :], op=MULT)
                    nc.vector.tensor_tensor(
                        out=var[:, :], in0=var[:, :], in1=msq[:, :], op=SUB)
                    nc.vector.tensor_scalar_add(var[:, :], var[:, :], EPS)
                    std = mp.tile([128, S], F32, name=f"std{s}_{ci}")
                    nc.scalar.activation(std[:, :], var[:, :], SQRT)
                    rinv = mp.tile([128, S], F32, name=f"rinv{s}_{ci}")
                    nc.vector.reciprocal(rinv[:, :], std[:, :])
                    scale = mp.tile([128, S], F32, name=f"scale{s}_{ci}")
                    nc.vector.tensor_tensor(
                        out=scale[:, :], in0=rinv[:, :], in1=gcm_t[s][ci][:, :],
                        op=MULT)
                    shift = mp.tile([128, S], F32, name=f"shift{s}_{ci}")
                    nc.vector.tensor_tensor(
                        out=shift[:, :], in0=mean[:, :], in1=scale[:, :], op=MULT)
                    nc.vector.tensor_tensor(
                        out=shift[:, :], in0=becm_t[s][ci][:, :], in1=shift[:, :],
                        op=SUB)
                    for dst, src in ((SC[s][ci], scale), (SH[s][ci], shift)):
                        prev = None
                        for sj in range(S):
                            o = dst if sj == S - 1 else scr[sj % 4]
                            if prev is None:
                                nc.vector.tensor_scalar_mul(
                                    o[:, :], mask_t[sj][:, :], src[:, sj:sj + 1])
                            else:
                                nc.vector.scalar_tensor_tensor(
                                    out=o[:, :], in0=mask_t[sj][:, :],
                                    scalar=src[:, sj:sj + 1], in1=prev[:, :],
                                    op0=MULT, op1=ADD)
                            prev = o

            # ================= stage 0 =================
            for tl in i1[0] + i2[0]:
                nc.vector.memset(tl[:, :], 0.0)
            for b in range(BSH):
                zA, zB, z0 = zAb[b % 4], zBb[b % 4], zt0[b % 4]
                nc.sync.dma_start(zA[0:128, 1:1 + T], Xd[b, 0:128, :])
                nc.sync.dma_start(zB[0:128, 1:1 + T], Xd[b, 128:256, :])
                nc.sync.dma_start(z0[0:15, 1:T], Xd[b, 256:CIN, 0:T - 1])
                nc.sync.dma_start(z0[32:47, 0:T], Xd[b, 256:CIN, :])
                nc.sync.dma_start(z0[64:79, 0:T - 1], Xd[b, 256:CIN, 1:T])
                conv_item(0, b)
            stats_stage(0)

            # ================= stages 1, 2 =================
            for s in (1, 2):
                for tl in i1[s] + i2[s]:
                    nc.vector.memset(tl[:, :], 0.0)
                for b in range(BSH):
                    h = 64 * (b % 2)
                    zA, zB = zAb[b % 4], zBb[b % 4]
                    zC, zt_ = zCb[(b // 2) % 2], ztl[b % 4]
                    nc.scalar.activation(
                        zA[0:128, 1:1 + T], yA[b][0:128, 0:T], GELU,
                        bias=SH[s - 1][0][:, b:b + 1],
                        scale=SC[s - 1][0][:, b:b + 1])
                    nc.scalar.activation(
                        zB[0:128, 1:1 + T], yB[b][0:128, 0:T], GELU,
                        bias=SH[s - 1][1][:, b:b + 1],
                        scale=SC[s - 1][1][:, b:b + 1])
                    nc.scalar.activation(
                        zC[h:h + 64, 1:1 + T], yC[b // 2][h:h + 64, 0:T], GELU,
                        bias=SH[s - 1][2][h:h + 64, b:b + 1],
                        scale=SC[s - 1][2][h:h + 64, b:b + 1])
                    # tail pack: rows 0-63 tap0 stream, rows 64-127 tap1
                    if h == 0:
                        nc.vector.tensor_copy(zt_[0:64, 0:T], zC[0:64, 0:T])
                        nc.vector.tensor_copy(zt_[64:128, 0:T], zC[0:64, 1:1 + T])
                    else:
                        nc.sync.dma_start(zt_[0:64, 0:T], zC[64:128, 0:T])
                        nc.vector.tensor_copy(zt_[64:128, 0:T], zC[64:128, 1:1 + T])
                    conv_item(s, b)
                stats_stage(s)

            # ================= final apply =================
            for b in range(BSH):
                h = 64 * (b % 2)
                zo = oA[b % 4]
                nc.scalar.activation(
                    zo[0:128, 0:T], yA[b][0:128, 0:T], GELU,
                    bias=SH[2][0][:, b:b + 1], scale=SC[2][0][:, b:b + 1])
                nc.sync.dma_start(OUTd[b, 0:128, :], zo[0:128, 0:T])
                zo = oB[b % 4]
                nc.scalar.activation(
                    zo[0:128, 0:T], yB[b][0:128, 0:T], GELU,
                    bias=SH[2][1][:, b:b + 1], scale=SC[2][1][:, b:b + 1])
                nc.sync.dma_start(OUTd[b, 128:256, :], zo[0:128, 0:T])
                zo = oC[b % 4]
                nc.scalar.activation(
                    zo[h:h + 64, 0:T], yC[b // 2][h:h + 64, 0:T], GELU,
                    bias=SH[2][2][h:h + 64, b:b + 1],
                    scale=SC[2][2][h:h + 64, b:b + 1])
                nc.sync.dma_start(OUTd[b, 256:COUT, :], zo[h:h + 64, 0:T])

    _split_multi_waits(nc, mybir)
    return nc


_CACHED = {}


def kernel(**inputs):
    X = np.asarray(inputs["X"], dtype=np.float32)
    subj = np.asarray(inputs["subject_idxs"], dtype=np.int32)
    w = [np.asarray(inputs[f"w{i}"], dtype=np.float32) for i in range(3)]
    g = [np.asarray(inputs[k], dtype=np.float32) for k in ("g0", "g1", "g2")]
    be = [np.asarray(inputs[k], dtype=np.float32) for k in ("be0", "be1", "be2")]
    # conv biases cancel inside per-subject BN (a uniform per-channel shift
    # is absorbed by the per-subject mean), so b0/b1/b2 are not needed.

    from concourse.bass_utils import run_bass_kernel_spmd

    trace = bool(int(os.environ.get("BASS_KERNEL_TRACE", "0")))
    if trace:
        _install_ntff_hook()

    if "nc" not in _CACHED:
        _CACHED["nc"] = _build_program()
    nc = _CACHED["nc"]

    # ---------------- host-side prep ----------------
    X16 = np.ascontiguousarray(X.astype(np.float16))
    wT = [[np.ascontiguousarray(w[s][:, :, tap].T) for tap in range(3)]
          for s in range(3)]
    wpk = np.zeros((23, 128, COUT), dtype=np.float16)
    for s in range(3):
        for kt in range(2):
            for tap in range(3):
                wpk[s * 6 + kt * 3 + tap] = wT[s][tap][kt * 128:(kt + 1) * 128]
    wpk[18][0:15] = wT[0][0][256:CIN]
    wpk[18][32:47] = wT[0][1][256:CIN]
    wpk[18][64:79] = wT[0][2][256:CIN]
    for s in (1, 2):
        base = 19 + 2 * (s - 1)
        wpk[base][0:64] = wT[s][0][256:COUT]
        wpk[base][64:128] = wT[s][1][256:COUT]
        wpk[base + 1][0:64] = wT[s][2][256:COUT]
        wpk[base + 1][64:128] = wT[s][2][256:COUT]

    cnt = np.maximum(
        np.bincount(subj, minlength=S).astype(np.float32) * float(T), 1.0)
    invc = np.broadcast_to((1.0 / cnt)[None, :], (128, S)).copy()

    gcm = np.zeros((3, 3, 128, S), np.float32)
    becm = np.zeros((3, 3, 128, S), np.float32)
    for s in range(3):
        for ci, (c0, c1) in enumerate(CT):
            m = c1 - c0
            gcm[s, ci, :m] = g[s].T[c0:c1]
            becm[s, ci, :m] = be[s].T[c0:c1]
            if ci == 2:  # duplicate so odd items (rows 64-127) see the same
                gcm[s, ci, 64:128] = g[s].T[c0:c1]
                becm[s, ci, 64:128] = be[s].T[c0:c1]

    shared = {"wpk": wpk, "invc": invc, "gcm": gcm, "becm": becm}

    in_maps = []
    for c in range(NCORES):
        sl = slice(c * BSH, (c + 1) * BSH)
        subj_c = subj[sl]
        masks = np.zeros((S, 128, BSH), dtype=np.float32)
        for bi in range(BSH):
            masks[subj_c[bi], :, bi] = 1.0
        m = dict(shared)
        m["xsh"] = X16[sl]
        m["masks"] = masks
        in_maps.append(m)

    res = run_bass_kernel_spmd(
        nc, in_maps, core_ids=list(range(NCORES)), trace=trace
    )
    if trace:
        _CACHED["exec_time_ns"] = res.exec_time_ns
        _CACHED["results_obj"] = res

    out = np.empty((B, COUT, T), dtype=np.float32)
    for c in range(NCORES):
        out[c * BSH:(c + 1) * BSH] = res.results[c]["out"].astype(np.float32)
    return out
